# revision 20
# baseline (speedup 1.0000x reference)
"""Causal attention (single head, S=4096, d=1024) on 8 TRN2 NeuronCores.

Sharding: core i computes output rows {i + 8m : m in 0..511} (strided
sequence-parallel Q). K/V projections are computed on contiguous chunks
[512i, 512(i+1)) and AllGathered in bf16. All matmuls run in bf16 with
f32 PSUM accumulation; softmax statistics in f32.

Per 128-row Q chunk b (rows span [1024b, 1024(b+1))), causal attention
needs exactly K[0 : 1024(b+1)] — identical on every core, so one SPMD
program serves all 8 cores; the diagonal-band mask (which depends on the
core index) is passed as a per-core input tensor.
"""

import numpy as np
import ml_dtypes

import concourse.bass as bass  # noqa: F401  (registers engines)
import concourse.mybir as mybir
from concourse import bacc, tile, masks
from concourse.bass_utils import run_bass_kernel_spmd

SEQ = 4096
D = 1024
N_CORES = 8
CORE_IDS = list(range(N_CORES))
QLOC = SEQ // N_CORES          # 512 q rows per core
NQCH = QLOC // 128             # 4 q chunks of 128 rows
BF16 = mybir.dt.bfloat16
F32 = mybir.dt.float32
MASK_VAL = -1.0e9
SM_SCALE = 1.0 / np.sqrt(np.float32(D))


def _emit_compute(nc, tc, pp, dp, cp_tiles, io, rep, variant="full"):
    """Emit one forward pass. `rep` uniquifies collective bounce bufs.

    variant: "proj" | "projag" | "noav" | "notrans" | "full" — truncated
    pipelines for phase attribution (debug outputs keep everything live).
    """
    ident, mask_sb = cp_tiles
    xq, xkv, wqT, wkT, wvT, out = io

    def dbg_consume(pool, aps, rows):
        """Cheaply consume `aps` (tiny slices) into `out` to defeat DCE."""
        o_dbg = pool.tile([128, 64], F32, tag="dbg", name=f"dbg{rep}_{rows}")
        for idx, ap_ in enumerate(aps[:8]):
            nc.vector.tensor_copy(o_dbg[:, 8 * idx:8 * (idx + 1)], ap_)
        nc.sync.dma_start(out[128 * (rows % 4):128 * (rows % 4) + 128, 0:64],
                          o_dbg[:])

    # DRAM bounce buffers for the single merged K+V collective.
    # Rank block layout: rows [0,1024) = K^T chunk [1024, 512];
    # rows [1024, 2048) = V chunk [512, 1024] flattened row-major.
    kv_ag_in = dp.tile([2 * D, QLOC], BF16, tag=f"kvagi{rep}",
                       name=f"kv_ag_in{rep}")
    kv_ag_out = dp.tile([N_CORES * 2 * D, QLOC], BF16, addr_space="Shared",
                        tag=f"kvago{rep}", name=f"kv_ag_out{rep}")

    with tc.tile_pool(name="persist", bufs=1) as pers:
        q_sb = pers.tile([128, 8, QLOC], BF16, name="q_sb")     # Q^T [d-chunk, q]
        kT_out = pers.tile([128, 8, QLOC], BF16, name="kT_out")  # own K^T chunk
        v_out = pers.tile([128, 4, D], BF16, name="v_out")       # own V chunk

        with tc.tile_pool(name="proj", bufs=1) as wp:
            xkv_sb = wp.tile([128, 8, QLOC], BF16, name="xkv_sb")
            xq_sb = wp.tile([128, 8, QLOC], BF16, name="xq_sb")
            wk_sb = wp.tile([128, 8, D], BF16, name="wk_sb")
            wv_sb = wp.tile([128, 8, D], BF16, name="wv_sb")
            wq_sb = wp.tile([128, 8, D], BF16, name="wq_sb")
            nc.sync.dma_start(xkv_sb[:], xkv.rearrange("(a p) s -> p a s", p=128))
            nc.sync.dma_start(wk_sb[:], wkT.rearrange("(a p) n -> p a n", p=128))
            nc.sync.dma_start(wv_sb[:], wvT.rearrange("(a p) n -> p a n", p=128))
            nc.sync.dma_start(xq_sb[:], xq.rearrange("(a p) s -> p a s", p=128))
            nc.sync.dma_start(wq_sb[:], wqT.rearrange("(a p) n -> p a n", p=128))

            # --- K^T chunk = Wk @ x_chunk^T : [1024 dout, 512 seq]
            for do in range(8):
                ps = pp.tile([128, QLOC], F32, tag="acc", bufs=4, name=f"ps_k{do}")
                for di in range(8):
                    nc.tensor.matmul(
                        ps[:], wk_sb[:, di, 128 * do:128 * (do + 1)],
                        xkv_sb[:, di, :], start=(di == 0), stop=(di == 7),
                    )
                nc.vector.tensor_copy(kT_out[:, do, :], ps[:])
            if variant != "proj":
                nc.sync.dma_start(
                    kv_ag_in[0:D, :].rearrange("(d p) s -> p d s", p=128),
                    kT_out[:]
                )

            # --- V chunk = x_chunk @ Wv^T : [512 seq, 1024 dout]
            for c in range(4):
                for h in range(2):
                    ps = pp.tile([128, 512], F32, tag="acc", bufs=4, name=f"ps_v{c}_{h}")
                    for di in range(8):
                        nc.tensor.matmul(
                            ps[:], xkv_sb[:, di, 128 * c:128 * (c + 1)],
                            wv_sb[:, di, 512 * h:512 * (h + 1)],
                            start=(di == 0), stop=(di == 7),
                        )
                    nc.vector.tensor_copy(v_out[:, c, 512 * h:512 * (h + 1)], ps[:])
            if variant != "proj":
                nc.sync.dma_start(
                    kv_ag_in[D:2 * D, :].rearrange(
                        "(c p two) s -> p c (two s)", p=128, two=2),
                    v_out[:]
                )
                nc.gpsimd.collective_compute(
                    "AllGather", mybir.AluOpType.bypass,
                    ins=[kv_ag_in.opt()], outs=[kv_ag_out.opt()],
                    replica_groups=[CORE_IDS],
                )

            # --- Q^T (strided rows) = Wq @ x_q^T
            for do in range(8):
                ps = pp.tile([128, QLOC], F32, tag="acc", bufs=4, name=f"ps_q{do}")
                for di in range(8):
                    nc.tensor.matmul(
                        ps[:], wq_sb[:, di, 128 * do:128 * (do + 1)],
                        xq_sb[:, di, :], start=(di == 0), stop=(di == 7),
                    )
                nc.vector.tensor_copy(q_sb[:, do, :], ps[:])

        if variant == "proj":
            dbg_consume(pers, [kT_out[:, 0, 0:8], v_out[:, 0, 0:8],
                               q_sb[:, 0, 0:8]], 0)
            return

        with (
            tc.tile_pool(name="kv", bufs=1) as kvp,
            tc.tile_pool(name="attn", bufs=2) as ap,
        ):
            # Load gathered K^T / V into SBUF (split DMAs for queue
            # parallelism, in the order attention consumes them).
            k_tiles, v_tiles = [], []
            for r in range(N_CORES):
                kt = kvp.tile([128, 8, 512], BF16, tag=f"k{r}", name=f"k_sb{r}")
                vt = kvp.tile([128, 4, D], BF16, tag=f"v{r}", name=f"v_sb{r}")
                k_tiles.append(kt)
                v_tiles.append(vt)
            for r in range(N_CORES):
                ksrc = kv_ag_out[2 * D * r:2 * D * r + D, :].rearrange(
                    "(d p) s -> p d s", p=128
                )
                vsrc = kv_ag_out[2 * D * r + D:2 * D * (r + 1), :].rearrange(
                    "(c p two) s -> p c (two s)", p=128, two=2
                )
                nc.sync.dma_start(k_tiles[r][:], ksrc[:])
                nc.sync.dma_start(v_tiles[r][:], vsrc[:])

            if variant == "agonly":
                # force AG completion with tiny reads, skip bulk loads
                tiny = ap.tile([128, 64], BF16, tag="tiny", name=f"tiny{rep}")
                for r in range(N_CORES):
                    nc.sync.dma_start(
                        tiny[:, 8 * r:8 * (r + 1)],
                        kv_ag_out[2 * D * r:2 * D * r + 128, 0:8])
                tiny2 = ap.tile([128, 8], BF16, tag="tiny2", name=f"tiny2{rep}")
                nc.sync.dma_start(tiny2[:], kv_ag_out[D:D + 128, 0:8])
                cast = ap.tile([128, 72], F32, tag="cast", name=f"cast{rep}")
                nc.vector.tensor_copy(cast[:, 0:64], tiny[:])
                nc.vector.tensor_copy(cast[:, 64:72], tiny2[:])
                nc.sync.dma_start(out[0:128, 0:72], cast[:])
                dbg_consume(ap, [q_sb[:, 0, 0:8]], 2)
                return

            if variant == "projag":
                dbg_consume(ap, [k_tiles[r][:, 0, 0:8] for r in range(4)]
                            + [v_tiles[r][:, 0, 0:8] for r in range(4)], 0)
                dbg_consume(ap, [k_tiles[r][:, 0, 0:8] for r in range(4, 8)]
                            + [v_tiles[r][:, 0, 0:8] for r in range(4, 8)], 1)
                dbg_consume(ap, [q_sb[:, 0, 0:8]], 2)
                return

            for b in range(NQCH):
                nkb = 2 * (b + 1)          # number of 512-wide k blocks
                klen = 512 * nkb
                a_sb = ap.tile([128, SEQ], BF16, tag="A", name=f"a_sb{b}")
                at_sb = ap.tile([128, SEQ], BF16, tag="AT", name=f"at_sb{b}")
                sums = ap.tile([128, 8], F32, tag="sums", name=f"sums{b}")

                # scores + exp (no max subtraction: |q.k|/32 is small)
                for kb in range(nkb):
                    ps_s = pp.tile([128, 512], F32, tag="acc", bufs=4, name=f"ps_s{b}_{kb}")
                    for di in range(8):
                        nc.tensor.matmul(
                            ps_s[:], q_sb[:, di, 128 * b:128 * (b + 1)],
                            k_tiles[kb][:, di, :],
                            start=(di == 0), stop=(di == 7),
                        )
                    if kb >= 2 * b:  # diagonal band: apply causal mask
                        j0 = 512 * (kb - 2 * b)
                        nc.vector.tensor_add(
                            ps_s[:], ps_s[:], mask_sb[:, j0:j0 + 512]
                        )
                    nc.scalar.activation(
                        a_sb[:, 512 * kb:512 * (kb + 1)], ps_s[:],
                        mybir.ActivationFunctionType.Exp,
                        scale=float(SM_SCALE),
                        accum_out=sums[:, kb:kb + 1],
                    )

                if variant == "noav":
                    dbg_consume(ap, [a_sb[:, 512 * kb:512 * kb + 8]
                                     for kb in range(nkb)] + [sums[:, 0:8]], b)
                    continue

                # transpose A in 128x128 tiles (PE) -> A^T for the AV matmul
                for kb in range(nkb):
                    ps_t = pp.tile([128, 512], BF16, tag="t", name=f"ps_t{b}_{kb}")
                    for cc in range(4):
                        nc.tensor.transpose(
                            ps_t[:, 128 * cc:128 * (cc + 1)],
                            a_sb[:, 512 * kb + 128 * cc:512 * kb + 128 * (cc + 1)],
                            ident[:],
                        )
                    nc.vector.tensor_copy(
                        at_sb[:, 512 * kb:512 * (kb + 1)], ps_t[:]
                    )

                if variant == "notrans":
                    dbg_consume(ap, [at_sb[:, 512 * kb:512 * kb + 8]
                                     for kb in range(nkb)] + [sums[:, 0:8]], b)
                    continue

                # row-sum reciprocal
                stot = ap.tile([128, 1], F32, tag="stot", name=f"stot{b}")
                rinv = ap.tile([128, 1], F32, tag="rinv", name=f"rinv{b}")
                nc.vector.reduce_sum(
                    out=stot[:], in_=sums[:, 0:nkb], axis=mybir.AxisListType.X
                )
                nc.vector.reciprocal(rinv[:], stot[:])

                # O = A @ V, then normalize rows by 1/sum
                o_sb = ap.tile([128, D], F32, tag="o", name=f"o_sb{b}")
                nkc = klen // 128
                for h in range(2):
                    ps_o = pp.tile([128, 512], F32, tag="o", name=f"ps_o{b}_{h}")
                    for kc in range(nkc):
                        nc.tensor.matmul(
                            ps_o[:], at_sb[:, 128 * kc:128 * (kc + 1)],
                            v_tiles[kc // 4][:, kc % 4, 512 * h:512 * (h + 1)],
                            start=(kc == 0), stop=(kc == nkc - 1),
                        )
                    nc.vector.tensor_scalar_mul(
                        o_sb[:, 512 * h:512 * (h + 1)], ps_o[:], rinv[:]
                    )
                nc.sync.dma_start(out[128 * b:128 * (b + 1), :], o_sb[:])


def build_nc(reps=1, variant="full"):
    nc = bacc.Bacc("TRN2", target_bir_lowering=False)

    xq = nc.dram_tensor("xq", [D, QLOC], BF16, kind="ExternalInput")
    xkv = nc.dram_tensor("xkv", [D, QLOC], BF16, kind="ExternalInput")
    wqT = nc.dram_tensor("wqT", [D, D], BF16, kind="ExternalInput")
    wkT = nc.dram_tensor("wkT", [D, D], BF16, kind="ExternalInput")
    wvT = nc.dram_tensor("wvT", [D, D], BF16, kind="ExternalInput")
    mask_in = nc.dram_tensor("mask", [128, 1024], F32, kind="ExternalInput")
    out = nc.dram_tensor("out", [QLOC, D], F32, kind="ExternalOutput")
    io = (xq, xkv, wqT, wkT, wvT, out)

    with tile.TileContext(nc) as tc:
        with (
            tc.tile_pool(name="const", bufs=1) as cp,
            tc.tile_pool(name="psum", bufs=2, space="PSUM") as pp,
            tc.tile_pool(name="dram", bufs=1, space="DRAM") as dp,
        ):
            ident = cp.tile([128, 128], BF16, name="ident")
            masks.make_identity(nc, ident[:])
            mask_sb = cp.tile([128, 1024], F32, name="mask_sb")
            nc.sync.dma_start(mask_sb[:], mask_in[:])
            for rep in range(reps):
                if rep > 0:
                    # serialize reps so the R-slope measures single-shot latency
                    tc.strict_bb_all_engine_barrier()
                _emit_compute(nc, tc, pp, dp, (ident, mask_sb), io, rep, variant)

    nc.compile()
    return nc


_NC_CACHE = None


def _get_nc():
    global _NC_CACHE
    if _NC_CACHE is None:
        _NC_CACHE = build_nc()
    return _NC_CACHE


def make_in_maps(x, Wq, Wk, Wv):
    x = np.asarray(x, dtype=np.float32)
    Wq = np.asarray(Wq, dtype=np.float32)
    Wk = np.asarray(Wk, dtype=np.float32)
    Wv = np.asarray(Wv, dtype=np.float32)

    bf = ml_dtypes.bfloat16
    xT = np.ascontiguousarray(x.T).astype(bf)          # [D, SEQ]
    wqT = np.ascontiguousarray(Wq.T).astype(bf)        # [D, D] (d_in major)
    wkT = np.ascontiguousarray(Wk.T).astype(bf)
    wvT = np.ascontiguousarray(Wv.T).astype(bf)

    p = np.arange(128)[:, None]
    j = np.arange(1024)[None, :]
    in_maps = []
    for i in CORE_IDS:
        mask_i = np.where(j <= 8 * p + i, 0.0, MASK_VAL).astype(np.float32)
        in_maps.append({
            "xq": np.ascontiguousarray(xT[:, i::N_CORES]),
            "xkv": np.ascontiguousarray(xT[:, QLOC * i:QLOC * (i + 1)]),
            "wqT": wqT, "wkT": wkT, "wvT": wvT,
            "mask": mask_i,
        })
    return in_maps


def assemble(results):
    out = np.empty((SEQ, D), dtype=np.float32)
    for i in CORE_IDS:
        out[i::N_CORES] = results[i]["out"]
    return out


def kernel(x, Wq, Wk, Wv):
    nc = _get_nc()
    in_maps = make_in_maps(x, Wq, Wk, Wv)
    res = run_bass_kernel_spmd(nc, in_maps, core_ids=CORE_IDS)
    return assemble(res.results)


if __name__ == "__main__":
    rng = np.random.RandomState(0)
    x = rng.randn(SEQ, D).astype(np.float32)
    s = 1.0 / np.sqrt(D)
    Wq = (rng.randn(D, D) * s).astype(np.float32)
    Wk = (rng.randn(D, D) * s).astype(np.float32)
    Wv = (rng.randn(D, D) * s).astype(np.float32)
    out = kernel(x, Wq, Wk, Wv)
    print("kernel ran, out shape", out.shape, "mean", out.mean())
